# revision 19
# baseline (speedup 1.0000x reference)
"""Trainium2 Bass kernel for nn_AutoregressiveLSA — fp8 DoubleRow version.

Math (complex, per batch b, one NeuronCore per batch element):
    Q  = WKQ @ E                       [2d, T]
    S  = E^H @ Q, keep i <= j          [T, T]
    outT[j] = sum_{i<=j} S[i,j] PT[i] * 2/max(j,1),  PT = (WPV @ E)^T

All matmuls run as fp8e4 (e4m3) in DoubleRow perf mode: one PE
instruction contracts TWO 128-chunks at 0.5 cycles/output-column (4x
the fp32r MAC rate).  Precision comes from a hi/lo split of every
operand (x ~ x_h + x_l, both e4m3; x_l*y_l dropped): per 128-chunk each
real product needs 3 fp8 pairings = 1.5 DR instructions, so a complex
Karatsuba product costs 2.25 free-columns/chunk vs 3.0 for fp32r.
Measured end-to-end rel err ~3e-3 (gate 2e-2).

Scale chain (powers of 2, folded into casts / final rho):
    E*4, WKQ^T*256, WPV^T*256 quantized on host.
    A1 psum = 1024*Q,  split scale 2^-7  -> Q'' = 8Q
    A2 psum = 1024*PT, split scale 2^-7  -> PT'' = 8PT
    B  psum = 32*S,    split scale 2^-9  -> S'' = S/16
    C  psum = S*PT/2,  rho2 = 2/max(j,1) applied via Act scale.

Engine constraints honored (probed on real TRN2): vector ops may read
at most ONE psum operand; Pool (gpsimd) runs SBUF-only tensor_tensor
(no psum, no scalar_tensor_tensor); Act does scaled copies (fp8 out ok).
Evacuation is fused into wide ops: psum banks ordered (M2, M1, M3) so
one 3W psum->sbuf copy + one dual-sub [re,tt] + pool im/sum + ONE 3W
Act h-cast + ONE 3W DVE stt l-split handle a whole complex site.
Phase B uses a host-negated E_im pack (nei) so its conjugated
recombination has the same (M1-M2', M3-M1-M2') form as the others.
"""

import numpy as np
import ml_dtypes

import concourse.bass as bass
import concourse.mybir as mybir
import concourse.tile as tile
from concourse import bacc
from concourse.bass_utils import run_bass_kernel_spmd
from concourse.alu_op_type import AluOpType

F32 = mybir.dt.float32
F8 = mybir.dt.float8e4
E4NP = ml_dtypes.float8_e4m3
DR = mybir.MatmulPerfMode.DoubleRow
COPY = mybir.ActivationFunctionType.Copy

B = 8
D2 = 1024
T = 2048
D = 512
P = 128
KC = D2 // P
MB = D2 // P
TB = T // P
A1W = 512
NJP = T // A1W
SPAN = 256
NSP = T // SPAN

CQ = float(2.0 ** -7)
CS = float(2.0 ** -9)


def pack_h0(t, fsl):
    """Slicer for h-first packs [P, K, 2(h,l), F] (E/S side)."""
    def f(k, kind):
        if kind == "hh":
            return t[:, 2 * k:2 * k + 2, 0, fsl]
        return t[:, k, :, fsl]
    return f


def pack_h1(t, fsl):
    """Slicer for l-first packs [P, K, 2(l,h), F] (W/Q/PT side)."""
    def f(k, kind):
        if kind == "hh":
            return t[:, 2 * k:2 * k + 2, 1, fsl]
        return t[:, k, :, fsl]
    return f


def dr_product(nc, bank, lhs, rhs, nk, leftover=None):
    nhh = nk // 2
    odd = nk % 2
    tot = nhh + nk + (1 if odd else 0)
    i = 0
    for kp in range(nhh):
        nc.tensor.matmul(bank, lhs(kp, "hh"), rhs(kp, "hh"),
                         start=(i == 0), stop=(i == tot - 1), perf_mode=DR)
        i += 1
    for k in range(nk):
        nc.tensor.matmul(bank, lhs(k, "x"), rhs(k, "x"),
                         start=(i == 0), stop=(i == tot - 1), perf_mode=DR)
        i += 1
    if odd:
        la, ra = leftover
        nc.tensor.matmul(bank, la, ra, start=(i == 0), stop=(i == tot - 1))


def build_module():
    nc = bacc.Bacc(target_bir_lowering=False, trn_type="TRN2")

    ep_r = nc.dram_tensor("ep_r", [P, KC, 2, T], F8, kind="ExternalInput")
    ep_i = nc.dram_tensor("ep_i", [P, KC, 2, T], F8, kind="ExternalInput")
    ep_ni = nc.dram_tensor("ep_ni", [P, KC, 2, T], F8, kind="ExternalInput")
    ep_s = nc.dram_tensor("ep_s", [P, KC, 2, T], F8, kind="ExternalInput")
    ep_d = nc.dram_tensor("ep_d", [P, KC, 2, T], F8, kind="ExternalInput")
    wp_r = nc.dram_tensor("wp_r", [KC, P, 2, D2], F8, kind="ExternalInput")
    wp_i = nc.dram_tensor("wp_i", [KC, P, 2, D2], F8, kind="ExternalInput")
    wp_s = nc.dram_tensor("wp_s", [KC, P, 2, D2], F8, kind="ExternalInput")
    vp_r = nc.dram_tensor("vp_r", [KC, P, 2, D], F8, kind="ExternalInput")
    vp_i = nc.dram_tensor("vp_i", [KC, P, 2, D], F8, kind="ExternalInput")
    vp_s = nc.dram_tensor("vp_s", [KC, P, 2, D], F8, kind="ExternalInput")
    trimask = nc.dram_tensor("trimask", [P, P], F32, kind="ExternalInput")
    rho2 = nc.dram_tensor("rho2", [P, TB], F32, kind="ExternalInput")
    outT_re = nc.dram_tensor("outT_re", [T, D], F32, kind="ExternalOutput")
    outT_im = nc.dram_tensor("outT_im", [T, D], F32, kind="ExternalOutput")

    _n = [0]

    def uid():
        _n[0] += 1
        return _n[0]

    with tile.TileContext(nc) as tc:
        with tc.tile_pool(name="dram", bufs=1, space="DRAM") as dram, \
             tc.tile_pool(name="erp", bufs=1) as erp, \
             tc.tile_pool(name="cst", bufs=1) as cst:
            q = dram.tile([MB, NJP, P, 6, A1W], F8, tag="q")
            pt = dram.tile([TB, P, 6, D], F8, tag="pt")
            s = dram.tile([TB, TB, P, 6, P], F8, tag="s")

            er = erp.tile([P, KC, 2, T], F8, tag="er")
            mask_sb = cst.tile([P, P], F32, tag="mask")
            rho_sb = cst.tile([P, TB], F32, tag="rho")

            def site_evac(pp, width, c, pk_h_ap, pk_l_ap, ev_pool, rc_pool,
                          masks=None):
                """Evacuate one complex site.

                pp: psum tile [P, 3, width] with banks (M2, M1, M3).
                pk_h_ap/pk_l_ap: output APs for h/l fp8 splits of
                (re, im, sum), or None to skip splits (phase C).
                Returns ev tile [P, 4, width] = (re, im, sum, tt).
                """
                n = uid()
                rc = rc_pool.tile([P, 3, width], F32, tag="rc", name=f"rc{n}")
                ev = ev_pool.tile([P, 4, width], F32, tag="ev", name=f"ev{n}")
                nc.scalar.activation(rc[:], pp[:], COPY)
                nc.vector.tensor_sub(ev[:, 0::3], rc[:, 1:3], rc[:, 0:2])
                nc.gpsimd.tensor_sub(ev[:, 1], ev[:, 3], rc[:, 0])
                if masks is not None:
                    for dsl in masks:
                        nc.vector.tensor_mul(ev[:, 0, dsl], ev[:, 0, dsl],
                                             mask_sb[:])
                        nc.vector.tensor_mul(ev[:, 1, dsl], ev[:, 1, dsl],
                                             mask_sb[:])
                if pk_h_ap is None:
                    return ev
                nc.gpsimd.tensor_add(ev[:, 2], ev[:, 0], ev[:, 1])
                nc.scalar.activation(pk_h_ap, ev[:, 0:3], COPY, scale=c)
                nc.vector.scalar_tensor_tensor(
                    out=pk_l_ap, in0=ev[:, 0:3], scalar=c, in1=pk_h_ap,
                    op0=AluOpType.mult, op1=AluOpType.subtract)
                return ev

            # =============== Phases A1 + A2 (merged psum scope) ===========
            with tc.tile_pool(name="eip", bufs=1) as eip, \
                 tc.tile_pool(name="esp", bufs=1) as esp:
                ei = eip.tile([P, KC, 2, T], F8, tag="ei")
                es = esp.tile([P, KC, 2, T], F8, tag="es")

                with tc.tile_pool(name="psA", bufs=2, space="PSUM") as psA, \
                     tc.tile_pool(name="rcA", bufs=2) as rcA, \
                     tc.tile_pool(name="evA", bufs=2) as evA, \
                     tc.tile_pool(name="pkA", bufs=2) as pkA, \
                     tc.tile_pool(name="wres", bufs=1) as wres:
                    wr = wres.tile([P, KC, 2, D2], F8, tag="wr")
                    wi = wres.tile([P, KC, 2, D2], F8, tag="wi")
                    ws = wres.tile([P, KC, 2, D2], F8, tag="ws")

                    with tc.tile_pool(name="vres", bufs=1) as vres:
                        vr = vres.tile([P, KC, 2, D], F8, tag="vr")
                        vi = vres.tile([P, KC, 2, D], F8, tag="vi")
                        vs = vres.tile([P, KC, 2, D], F8, tag="vs")

                        def eslice(dst, srct, jp):
                            js = bass.ds(jp * A1W, A1W)
                            nc.sync.dma_start(dst[:, :, :, js],
                                              srct[:, :, :, js])

                        nc.sync.dma_start(
                            vr[:], vp_r[:].rearrange("k p s m -> p k s m"))
                        nc.sync.dma_start(
                            vi[:], vp_i[:].rearrange("k p s m -> p k s m"))
                        nc.sync.dma_start(
                            vs[:], vp_s[:].rearrange("k p s m -> p k s m"))
                        for _jp in range(NJP):
                            eslice(er, ep_r, _jp)
                            eslice(ei, ep_i, _jp)
                            eslice(es, ep_s, _jp)
                        nc.sync.dma_start(
                            wr[:], wp_r[:].rearrange("k p s m -> p k s m"))
                        nc.sync.dma_start(
                            wi[:], wp_i[:].rearrange("k p s m -> p k s m"))
                        nc.sync.dma_start(
                            ws[:], wp_s[:].rearrange("k p s m -> p k s m"))
                        nc.sync.dma_start(mask_sb[:], trimask[:])
                        nc.sync.dma_start(rho_sb[:], rho2[:])

                        # -------- A2 (first): PT = (WPV @ E)^T --------
                        vd = bass.ds(0, D)
                        for tb in range(TB):
                            tbs = bass.ts(tb, P)
                            n = uid()
                            pp = psA.tile([P, 3, D], F32, tag="pp",
                                          name=f"pp{n}")
                            dr_product(nc, pp[:, 0], pack_h0(ei, tbs),
                                       pack_h1(vi, vd), KC)
                            dr_product(nc, pp[:, 1], pack_h0(er, tbs),
                                       pack_h1(vr, vd), KC)
                            dr_product(nc, pp[:, 2], pack_h0(es, tbs),
                                       pack_h1(vs, vd), KC)
                            ppk = pkA.tile([P, 6, D], F8, tag="pk",
                                           name=f"ppk{n}")
                            site_evac(pp, D, CQ, ppk[:, 1::2], ppk[:, 0::2],
                                      evA, rcA)
                            nc.scalar.dma_start(pt[tb], ppk[:])

                    # -------- A1: Q = WKQ @ E --------
                    for jp in range(NJP):
                        js = bass.ds(jp * A1W, A1W)
                        for m in range(MB):
                            ms = bass.ts(m, P)
                            n = uid()
                            pp = psA.tile([P, 3, A1W], F32, tag="pp",
                                          name=f"pp{n}")
                            dr_product(nc, pp[:, 0], pack_h1(wi, ms),
                                       pack_h0(ei, js), KC)
                            dr_product(nc, pp[:, 1], pack_h1(wr, ms),
                                       pack_h0(er, js), KC)
                            dr_product(nc, pp[:, 2], pack_h1(ws, ms),
                                       pack_h0(es, js), KC)
                            qpk = pkA.tile([P, 6, A1W], F8, tag="pk",
                                           name=f"qpk{n}")
                            site_evac(pp, A1W, CQ,
                                      qpk[:, 1::2], qpk[:, 0::2],
                                      evA, rcA)
                            nc.scalar.dma_start(q[m, jp], qpk[:])

            # =============== Phase B: S = E^H Q (upper tri) ===============
            with tc.tile_pool(name="ptp", bufs=1) as ptpp:
                ptr = ptpp.tile([P, TB, 2, D], F8, tag="ptr")
                pti = ptpp.tile([P, TB, 2, D], F8, tag="pti")

                with tc.tile_pool(name="edp", bufs=1) as edp, \
                     tc.tile_pool(name="qsbp", bufs=2) as qsbp, \
                     tc.tile_pool(name="psB", bufs=2, space="PSUM") as psB, \
                     tc.tile_pool(name="rcB", bufs=2) as rcB, \
                     tc.tile_pool(name="evB", bufs=2) as evB, \
                     tc.tile_pool(name="spkp", bufs=3) as spkp:
                    nei = edp.tile([P, KC, 2, T], F8, tag="nei")
                    ed = edp.tile([P, KC, 2, T], F8, tag="ed")
                    qsb_tiles = {}

                    def load_qsb(psp):
                        t = qsbp.tile([P, MB, 6, A1W], F8, tag="qsb",
                                      name=f"qsb{psp}")
                        nc.sync.dma_start(
                            t[:], q[:, psp].rearrange("m p v t -> p m v t"))
                        qsb_tiles[psp] = t

                    load_qsb(0)
                    for _jp in range(NJP):
                        js = bass.ds(_jp * A1W, A1W)
                        nc.sync.dma_start(nei[:, :, :, js], ep_ni[:, :, :, js])
                        nc.sync.dma_start(ed[:, :, :, js], ep_d[:, :, :, js])
                    nc.sync.dma_start(
                        ptr[:], pt[:, :, 0:2].rearrange("t p v d -> p t v d"))
                    nc.sync.dma_start(
                        pti[:], pt[:, :, 2:4].rearrange("t p v d -> p t v d"))
                    for psp in range(NJP):
                        if psp + 1 < NJP:
                            load_qsb(psp + 1)
                        qsb = qsb_tiles.pop(psp)
                        for half in range(2):
                            sp = 2 * psp + half
                            jsl = bass.ds(half * SPAN, SPAN)

                            def rhs_q(vb):
                                def f(k, kind):
                                    if kind == "hh":
                                        return qsb[:, 2 * k:2 * k + 2,
                                                   vb + 1, jsl]
                                    return qsb[:, k, vb:vb + 2, jsl]
                                return f

                            for ib in range(2 * sp + 2):
                                ibs = bass.ts(ib, P)
                                n = uid()
                                pp = psB.tile([P, 3, SPAN], F32, tag="pp",
                                              name=f"pp{n}")
                                dr_product(nc, pp[:, 0], pack_h0(nei, ibs),
                                           rhs_q(2), KC)
                                dr_product(nc, pp[:, 1], pack_h0(er, ibs),
                                           rhs_q(0), KC)
                                dr_product(nc, pp[:, 2], pack_h0(ed, ibs),
                                           rhs_q(4), KC)
                                masks = [bass.ds(jh * P, P) for jh in range(2)
                                         if ib == 2 * sp + jh]
                                spk = spkp.tile([P, 2, 6, P], F8, tag="spk",
                                                name=f"spk{n}")
                                h_ap = spk[:, :, 0::2].rearrange(
                                    "p a v j -> p v a j")
                                l_ap = spk[:, :, 1::2].rearrange(
                                    "p a v j -> p v a j")
                                site_evac(pp, SPAN, CS, h_ap, l_ap,
                                          evB, rcB, masks=masks)
                                for jh in range(2):
                                    jb = 2 * sp + jh
                                    if ib <= jb:
                                        nc.scalar.dma_start(s[ib, jb],
                                                            spk[:, jh])

                # =============== Phase C (descending jb) ===============
                with tc.tile_pool(name="ptsp", bufs=1) as ptsp, \
                     tc.tile_pool(name="sstp", bufs=3) as sstp, \
                     tc.tile_pool(name="psC", bufs=2, space="PSUM") as psC, \
                     tc.tile_pool(name="rcC", bufs=2) as rcC, \
                     tc.tile_pool(name="evC", bufs=2) as evC, \
                     tc.tile_pool(name="out4", bufs=3) as out4:
                    pts = ptsp.tile([P, TB, 2, D], F8, tag="pts")
                    nc.sync.dma_start(
                        pts[:], pt[:, :, 4:6].rearrange("t p v d -> p t v d"))
                    vd = bass.ds(0, D)
                    sst_tiles = {}

                    def load_sst(jb):
                        t = sstp.tile([P, TB, 6, P], F8, tag="sst",
                                      name=f"sst{jb}")[:, :jb + 1]
                        nc.sync.dma_start(
                            t[:], s[:jb + 1, jb].rearrange(
                                "i p v j -> p i v j"))
                        sst_tiles[jb] = t

                    load_sst(0)
                    load_sst(1)
                    for jb in range(TB):
                        jbs = bass.ts(jb, P)
                        nk = jb + 1
                        if jb + 2 < TB:
                            load_sst(jb + 2)
                        sst = sst_tiles.pop(jb)

                        def lhs_s(vb):
                            def f(k, kind):
                                if kind == "hh":
                                    return sst[:, 2 * k:2 * k + 2, vb, :]
                                return sst[:, k, vb:vb + 2, :]
                            return f

                        n = uid()
                        pp = psC.tile([P, 3, D], F32, tag="pp", name=f"pp{n}")
                        kl = nk - 1
                        dr_product(nc, pp[:, 0], lhs_s(2), pack_h1(pti, vd),
                                   nk, leftover=(sst[:, kl, 2, :],
                                                 pti[:, kl, 1, vd]))
                        dr_product(nc, pp[:, 1], lhs_s(0), pack_h1(ptr, vd),
                                   nk, leftover=(sst[:, kl, 0, :],
                                                 ptr[:, kl, 1, vd]))
                        dr_product(nc, pp[:, 2], lhs_s(4), pack_h1(pts, vd),
                                   nk, leftover=(sst[:, kl, 4, :],
                                                 pts[:, kl, 1, vd]))
                        ev = site_evac(pp, D, None, None, None, evC, rcC)
                        oo = out4.tile([P, 2, D], F32, tag="oo",
                                       name=f"oo{jb}")
                        nc.scalar.activation(oo[:], ev[:, 0:2], COPY,
                                             scale=rho_sb[:, jb:jb + 1])
                        nc.scalar.dma_start(outT_re[jbs, :], oo[:, 0])
                        nc.scalar.dma_start(outT_im[jbs, :], oo[:, 1])

    nc.compile()
    return nc


_NC_CACHE = None


def _get_module():
    global _NC_CACHE
    if _NC_CACHE is None:
        _NC_CACHE = build_module()
    return _NC_CACHE


def _split(x):
    h = x.astype(E4NP)
    l = (x - h.astype(np.float32)).astype(E4NP)
    return h, l


def _pack(x, hfirst):
    """x [D2, F] f32 -> fp8 pack: [P, KC, 2, F] (E, h-first) or
    [KC, P, 2, F] (weights, l-first)."""
    h, l = _split(x)
    F = x.shape[1]
    if hfirst:
        out = np.empty((P, KC, 2, F), E4NP)
        out[:, :, 0] = h.reshape(KC, P, F).transpose(1, 0, 2)
        out[:, :, 1] = l.reshape(KC, P, F).transpose(1, 0, 2)
    else:
        out = np.empty((KC, P, 2, F), E4NP)
        out[:, :, 1] = h.reshape(KC, P, F)
        out[:, :, 0] = l.reshape(KC, P, F)
    return out


def prep_shared(WKQ_re, WKQ_im, WPV_re, WPV_im):
    wr = np.ascontiguousarray(WKQ_re.T) * 256.0
    wi = np.ascontiguousarray(WKQ_im.T) * 256.0
    vr = np.ascontiguousarray(WPV_re.T) * 256.0
    vi = np.ascontiguousarray(WPV_im.T) * 256.0
    shared = {
        "wp_r": _pack(wr, False), "wp_i": _pack(wi, False),
        "wp_s": _pack(wr + wi, False),
        "vp_r": _pack(vr, False), "vp_i": _pack(vi, False),
        "vp_s": _pack(vr + vi, False),
        "trimask": np.triu(np.ones((P, P), np.float32)),
    }
    j = np.arange(T, dtype=np.float32)
    rho = 2.0 / np.maximum(j, 1.0)
    shared["rho2"] = np.ascontiguousarray(rho.reshape(TB, P).T)
    return shared


def kernel(E_re, E_im, WKQ_re, WKQ_im, WPV_re, WPV_im):
    E_re = np.asarray(E_re, dtype=np.float32)
    E_im = np.asarray(E_im, dtype=np.float32)
    shared = prep_shared(np.asarray(WKQ_re, np.float32),
                         np.asarray(WKQ_im, np.float32),
                         np.asarray(WPV_re, np.float32),
                         np.asarray(WPV_im, np.float32))
    in_maps = []
    for b in range(B):
        er = E_re[b] * 4.0
        ei = E_im[b] * 4.0
        m = dict(shared)
        m["ep_r"] = _pack(er, True)
        m["ep_i"] = _pack(ei, True)
        m["ep_ni"] = _pack(-ei, True)
        m["ep_s"] = _pack(er + ei, True)
        m["ep_d"] = _pack(er - ei, True)
        in_maps.append(m)

    nc = _get_module()
    res = run_bass_kernel_spmd(nc, in_maps, core_ids=list(range(B)))

    out = np.empty((B, D, T - 2), dtype=np.complex64)
    for b in range(B):
        r = res.results[b]["outT_re"]  # [T, D]
        i = res.results[b]["outT_im"]
        full = (r + 1j * i.astype(np.complex64)).T  # [D, T]
        out[b] = full[:, 1:T - 1]
    return out
